# revision 1
# baseline (speedup 1.0000x reference)
"""GCN (2-layer, symmetric-normalized, self-loops) on 8 TRN2 NeuronCores.

Math (reference):
    A_hat = D^-1/2 (A + I) D^-1/2        (deg over dst incl. self-loops)
    h1    = relu(A_hat @ (x @ W1) + b1)
    out   = log_softmax(A_hat @ h1 @ W2 + b2)

Device decomposition (nodes sharded by range across 8 cores, 3 launches):
    K1: ut   = dinv * (x @ W1)                       [per-core shard]
    K2: ht   = dinv * relu(dinv * ((A+I) @ ut) + b1) [gather ut table]
    K3: out  = log_softmax((dinv * ((A+I) @ ht)) @ W2 + b2)
Host concatenates shard outputs between launches (index structures are
pure functions of edge_index and are built host-side).

The (A+I) @ T aggregation per core: edges (grouped by 256-dst-node
"pair" windows x int16 source chunks, padded to 128-edge blocks,
uniform across cores for SPMD) are fetched with dma_gather and
scatter-summed into per-window PSUM accumulators via one-hot selection
matrices (is_equal against an iota row) on the tensor engine.
"""

import math
import os
import sys
import types

import numpy as np

# ---------------------------------------------------------------- sizes
N = 100000
E = 1600000
F_IN = 256
H = 64
C = 16
NCORE = 8
P = 128
CHUNK = 25000            # int16-addressable source chunk (balanced)
TRACE = bool(int(os.environ.get("BASS_GCN_TRACE", "0")))

LAST_EXEC_NS = []        # per-launch exec time (filled when TRACE)


def _derived():
    ncn = N // NCORE
    padn = ((ncn + 255) // 256) * 256
    nwin = padn // P
    npair = nwin // 2
    nchunk = (N + CHUNK - 1) // CHUNK
    return ncn, padn, nwin, npair, nchunk


# ------------------------------------------------------- ntff shim (opt)
def _install_ntff_shim():
    try:
        if "antenv.axon_hooks" in sys.modules:
            return True
        sys.path.insert(0, "/root/.axon_site/trn_agent_boot")
        from trn_boot import _ntff_profile_via_ctypes  # type: ignore

        mod = types.ModuleType("antenv.axon_hooks")
        holder = [None]
        mod.set_axon_ntff_profile_hook = lambda h: holder.__setitem__(0, h)
        mod.get_axon_ntff_profile_hook = lambda: holder[0]
        sys.modules["antenv.axon_hooks"] = mod
        import antenv

        antenv.axon_hooks = mod
        mod.set_axon_ntff_profile_hook(
            _ntff_profile_via_ctypes("/opt/axon/libaxon_pjrt.so")
        )
        return True
    except Exception:
        return False


# ------------------------------------------------------------ host plan
def _build_plan(edge_index):
    """Index structures for the per-core edge aggregation.

    Returns dict with:
      S        [npair*nchunk] int  padded edge count per segment (uniform)
      idxw     [NCORE][128, sum(S)/16] int16   wrapped gather indices
      slotcols [NCORE][128, sum(S)/128] f32    slot-in-pair per block col
      dinv_w   [NCORE][128, nwin] f32          dinv per window column
      dinv     [N] f32
    """
    ncn, padn, nwin, npair, nchunk = _derived()
    nseg = npair * nchunk

    loop = np.arange(N, dtype=edge_index.dtype)
    src = np.concatenate([edge_index[0], loop]).astype(np.int64)
    dst = np.concatenate([edge_index[1], loop]).astype(np.int64)
    deg = np.bincount(dst, minlength=N).astype(np.float64)
    dinv = (1.0 / np.sqrt(deg)).astype(np.float32)

    per_core = []
    cnts = np.zeros((NCORE, nseg), np.int64)
    for c in range(NCORE):
        lo = c * ncn
        m = (dst >= lo) & (dst < lo + ncn)
        s = src[m]
        d = dst[m] - lo
        pair = d >> 8
        chunk = s // CHUNK
        segid = pair * nchunk + chunk
        order = np.argsort(segid, kind="stable")
        s, d, segid = s[order], d[order], segid[order]
        cnts[c] = np.bincount(segid, minlength=nseg)
        per_core.append((s, d, segid))

    S = 128 * ((cnts.max(axis=0) + 127) // 128)  # [nseg]
    off = np.zeros(nseg + 1, np.int64)
    off[1:] = np.cumsum(S)
    total = int(off[-1])

    idxw_l, slot_l, dinvw_l = [], [], []
    for c in range(NCORE):
        s, d, segid = per_core[c]
        idx16 = np.zeros(total, np.int16)
        slot = np.full(total, 999.0, np.float32)
        seg_start = np.searchsorted(segid, np.arange(nseg))
        pos = off[segid] + (np.arange(len(s)) - seg_start[segid])
        idx16[pos] = (s % CHUNK).astype(np.int16)
        slot[pos] = (d & 255).astype(np.float32)
        # wrap indices per segment: [S] -> [16, S/16] -> tile to 128 rows
        cols16 = []
        slotcols = []
        for g in range(nseg):
            a, b = int(off[g]), int(off[g + 1])
            if a == b:
                continue
            seg = idx16[a:b]
            cols16.append(np.tile(seg.reshape(-1, 16).T, (8, 1)))
            slotcols.append(slot[a:b].reshape(-1, P).T)
        idxw_l.append(np.ascontiguousarray(np.concatenate(cols16, axis=1)))
        slot_l.append(np.ascontiguousarray(np.concatenate(slotcols, axis=1)))
        dv = np.zeros((P, nwin), np.float32)
        valid = np.arange(padn) < ncn
        dvfull = np.zeros(padn, np.float32)
        dvfull[:ncn] = dinv[c * ncn : c * ncn + ncn]
        dv[:, :] = dvfull.reshape(nwin, P).T * valid.reshape(nwin, P).T
        dinvw_l.append(dv)

    return {
        "S": S,
        "idxw": idxw_l,
        "slotcols": slot_l,
        "dinv_w": dinvw_l,
        "dinv": dinv,
    }


# --------------------------------------------------------- bass builders
def _bass_mods():
    import concourse.bass as bass
    import concourse.bacc as bacc
    import concourse.tile as tile
    import concourse.mybir as mybir
    from concourse import library_config
    from concourse.masks import make_identity

    return bass, bacc, tile, mybir, library_config, make_identity


def _build_k1():
    """ut[PADN, H] = dinv_col * (x @ W1) per core. Input xT [F_IN, PADN]."""
    bass, bacc, tile, mybir, libcfg, make_identity = _bass_mods()
    ncn, padn, nwin, npair, nchunk = _derived()
    f32 = mybir.dt.float32

    nc = bacc.Bacc("TRN2", target_bir_lowering=False, debug=False,
                   num_devices=NCORE)
    xT = nc.dram_tensor("xT", [F_IN, padn], f32, kind="ExternalInput").ap()
    w1 = nc.dram_tensor("w1", [F_IN, H], f32, kind="ExternalInput").ap()
    dinvd = nc.dram_tensor("dinvw", [P, nwin], f32, kind="ExternalInput").ap()
    ut = nc.dram_tensor("ut", [padn, H], f32, kind="ExternalOutput").ap()

    kf = F_IN // P
    with tile.TileContext(nc) as tc:
        with (
            tc.tile_pool(name="const", bufs=1) as constp,
            tc.tile_pool(name="xin", bufs=4) as xp,
            tc.tile_pool(name="ps", bufs=2, space="PSUM") as psump,
            tc.tile_pool(name="wk", bufs=4) as wp,
        ):
            w1_s = constp.tile([P, kf * H], f32)
            for k in range(kf):
                nc.sync.dma_start(w1_s[:, k * H : (k + 1) * H],
                                  w1[k * P : (k + 1) * P, :])
            dinv_s = constp.tile([P, nwin], f32)
            nc.sync.dma_start(dinv_s[:], dinvd[:, :])
            ident = constp.tile([H, H], f32)
            make_identity(nc, ident[:])

            for t in range(nwin):
                up = psump.tile([H, P], f32, tag="up", bufs=2)
                for k in range(kf):
                    xt = xp.tile([P, P], f32, tag="xt")
                    nc.sync.dma_start(
                        xt[:], xT[k * P : (k + 1) * P, t * P : (t + 1) * P]
                    )
                    nc.tensor.matmul(
                        up[:], lhsT=w1_s[:, k * H : (k + 1) * H], rhs=xt[:],
                        start=(k == 0), stop=(k == kf - 1),
                    )
                uts = wp.tile([H, P], f32, tag="uts")
                nc.vector.tensor_copy(uts[:], up[:])
                u2p = psump.tile([P, H], f32, tag="u2p", bufs=2)
                nc.tensor.transpose(u2p[:], uts[:], ident[:])
                uo = wp.tile([P, H], f32, tag="uo")
                nc.vector.tensor_scalar_mul(uo[:], u2p[:], dinv_s[:, t : t + 1])
                nc.sync.dma_start(ut[t * P : (t + 1) * P, :], uo[:])
    nc.compile()
    return nc


def _agg_pairs(nc, tc, mybir, table, idx_s, slot_s, iota_s, S, drain_fn,
               pools):
    """Shared aggregation loop: for each pair, gather+scatter-sum edges
    into two window PSUM tiles, then call drain_fn(pr, ps0, ps1)."""
    f32 = mybir.dt.float32
    ncn, padn, nwin, npair, nchunk = _derived()
    gatp, selp, psump = pools
    off16 = 0
    mmcol = 0
    for pr in range(npair):
        ps0 = psump.tile([P, H], f32, tag="ps0", bufs=2, name="ps0")
        ps1 = psump.tile([P, H], f32, tag="ps1", bufs=2, name="ps1")
        blocks = []
        for ch in range(nchunk):
            s = int(S[pr * nchunk + ch])
            if s == 0:
                continue
            gat = gatp.tile([P, s // P, H], f32, tag="gat", name="gat")
            nc.gpsimd.dma_gather(
                gat[:],
                table[ch * CHUNK : min(N, (ch + 1) * CHUNK), :],
                idx_s[:, off16 : off16 + s // 16],
                s, s, H, elem_step=H, single_packet=False,
            )
            off16 += s // 16
            blocks.append((gat, s // P))
        nbt = sum(nb for _, nb in blocks)
        bi = 0
        for gat, nb in blocks:
            for b in range(nb):
                sel2 = selp.tile([P, 2 * P], f32, tag="sel2", name="sel2")
                nc.vector.tensor_tensor(
                    out=sel2[:],
                    in0=slot_s[:, mmcol : mmcol + 1].to_broadcast([P, 2 * P]),
                    in1=iota_s[:],
                    op=mybir.AluOpType.is_equal,
                )
                nc.tensor.matmul(
                    ps0[:], lhsT=sel2[:, :P], rhs=gat[:, b, :],
                    start=(bi == 0), stop=(bi == nbt - 1),
                )
                nc.tensor.matmul(
                    ps1[:], lhsT=sel2[:, P:], rhs=gat[:, b, :],
                    start=(bi == 0), stop=(bi == nbt - 1),
                )
                mmcol += 1
                bi += 1
        drain_fn(pr, ps0, ps1)


def _build_k2(S, idx_cols, slot_cols):
    """ht[PADN, H] = dinv*relu(dinv*agg(ut) + b1) per core."""
    bass, bacc, tile, mybir, libcfg, make_identity = _bass_mods()
    ncn, padn, nwin, npair, nchunk = _derived()
    f32 = mybir.dt.float32

    nc = bacc.Bacc("TRN2", target_bir_lowering=False, debug=False,
                   num_devices=NCORE)
    table = nc.dram_tensor("table", [N, H], f32, kind="ExternalInput").ap()
    idxd = nc.dram_tensor("idx", [P, idx_cols], mybir.dt.int16,
                          kind="ExternalInput").ap()
    slotd = nc.dram_tensor("slot", [P, slot_cols], f32,
                           kind="ExternalInput").ap()
    iotad = nc.dram_tensor("iota", [P, 2 * P], f32, kind="ExternalInput").ap()
    dinvd = nc.dram_tensor("dinvw", [P, nwin], f32, kind="ExternalInput").ap()
    b1d = nc.dram_tensor("b1rep", [P, H], f32, kind="ExternalInput").ap()
    ht = nc.dram_tensor("ht", [padn, H], f32, kind="ExternalOutput").ap()

    with tile.TileContext(nc) as tc:
        with (
            tc.tile_pool(name="const", bufs=1) as constp,
            tc.tile_pool(name="gat", bufs=4) as gatp,
            tc.tile_pool(name="sel", bufs=4) as selp,
            tc.tile_pool(name="ps", bufs=1, space="PSUM") as psump,
            tc.tile_pool(name="wk", bufs=4) as wp,
        ):
            with tc.tile_critical():
                nc.gpsimd.load_library(libcfg.mlp)
            idx_s = constp.tile([P, idx_cols], mybir.dt.int16)
            nc.sync.dma_start(idx_s[:], idxd[:, :])
            slot_s = constp.tile([P, slot_cols], f32)
            nc.sync.dma_start(slot_s[:], slotd[:, :])
            iota_s = constp.tile([P, 2 * P], f32)
            nc.sync.dma_start(iota_s[:], iotad[:, :])
            dinv_s = constp.tile([P, nwin], f32)
            nc.sync.dma_start(dinv_s[:], dinvd[:, :])
            b1_s = constp.tile([P, H], f32)
            nc.sync.dma_start(b1_s[:], b1d[:, :])

            def drain(pr, ps0, ps1):
                for wi, ps in ((2 * pr, ps0), (2 * pr + 1, ps1)):
                    t1 = wp.tile([P, H], f32, tag="t1", name="t1")
                    nc.vector.tensor_scalar_mul(
                        t1[:], ps[:], dinv_s[:, wi : wi + 1]
                    )
                    t2 = wp.tile([P, H], f32, tag="t2", name="t2")
                    nc.vector.tensor_tensor(
                        t2[:], t1[:], b1_s[:], op=mybir.AluOpType.add
                    )
                    nc.vector.tensor_scalar_max(t2[:], t2[:], 0.0)
                    nc.vector.tensor_scalar_mul(
                        t2[:], t2[:], dinv_s[:, wi : wi + 1]
                    )
                    nc.sync.dma_start(ht[wi * P : (wi + 1) * P, :], t2[:])

            _agg_pairs(nc, tc, mybir, table, idx_s, slot_s, iota_s, S, drain,
                       (gatp, selp, psump))
    nc.compile()
    return nc


def _build_k3(S, idx_cols, slot_cols):
    """out[PADN, C] = log_softmax((dinv*agg(ht)) @ W2 + b2) per core."""
    bass, bacc, tile, mybir, libcfg, make_identity = _bass_mods()
    ncn, padn, nwin, npair, nchunk = _derived()
    f32 = mybir.dt.float32

    nc = bacc.Bacc("TRN2", target_bir_lowering=False, debug=False,
                   num_devices=NCORE)
    table = nc.dram_tensor("table", [N, H], f32, kind="ExternalInput").ap()
    idxd = nc.dram_tensor("idx", [P, idx_cols], mybir.dt.int16,
                          kind="ExternalInput").ap()
    slotd = nc.dram_tensor("slot", [P, slot_cols], f32,
                           kind="ExternalInput").ap()
    iotad = nc.dram_tensor("iota", [P, 2 * P], f32, kind="ExternalInput").ap()
    dinvd = nc.dram_tensor("dinvw", [P, nwin], f32, kind="ExternalInput").ap()
    w2d = nc.dram_tensor("w2", [H, C], f32, kind="ExternalInput").ap()
    b2d = nc.dram_tensor("b2rep", [P, C], f32, kind="ExternalInput").ap()
    outd = nc.dram_tensor("out", [padn, C], f32, kind="ExternalOutput").ap()

    with tile.TileContext(nc) as tc:
        with (
            tc.tile_pool(name="const", bufs=1) as constp,
            tc.tile_pool(name="gat", bufs=4) as gatp,
            tc.tile_pool(name="sel", bufs=4) as selp,
            tc.tile_pool(name="ps", bufs=1, space="PSUM") as psump,
            tc.tile_pool(name="wk", bufs=4) as wp,
        ):
            with tc.tile_critical():
                nc.gpsimd.load_library(libcfg.mlp)
            idx_s = constp.tile([P, idx_cols], mybir.dt.int16)
            nc.sync.dma_start(idx_s[:], idxd[:, :])
            slot_s = constp.tile([P, slot_cols], f32)
            nc.sync.dma_start(slot_s[:], slotd[:, :])
            iota_s = constp.tile([P, 2 * P], f32)
            nc.sync.dma_start(iota_s[:], iotad[:, :])
            dinv_s = constp.tile([P, nwin], f32)
            nc.sync.dma_start(dinv_s[:], dinvd[:, :])
            w2_s = constp.tile([H, C], f32)
            nc.sync.dma_start(w2_s[:], w2d[:, :])
            b2_s = constp.tile([P, C], f32)
            nc.sync.dma_start(b2_s[:], b2d[:, :])
            ident = constp.tile([P, P], f32)
            make_identity(nc, ident[:])

            def drain(pr, ps0, ps1):
                for wi, ps in ((2 * pr, ps0), (2 * pr + 1, ps1)):
                    t1 = wp.tile([P, H], f32, tag="t1", name="t1")
                    nc.vector.tensor_scalar_mul(
                        t1[:], ps[:], dinv_s[:, wi : wi + 1]
                    )
                    t1T_p = psump.tile([H, P], f32, tag="t1T", bufs=1,
                                       name="t1T")
                    nc.tensor.transpose(t1T_p[:], t1[:], ident[:])
                    t1T = wp.tile([H, P], f32, tag="t1Ts", name="t1Ts")
                    nc.vector.tensor_copy(t1T[:], t1T_p[:])
                    yT_p = psump.tile([C, P], f32, tag="yT", bufs=1, name="yT")
                    nc.tensor.matmul(yT_p[:], lhsT=w2_s[:], rhs=t1T[:],
                                     start=True, stop=True)
                    yT = wp.tile([C, P], f32, tag="yTs", name="yTs")
                    nc.vector.tensor_copy(yT[:], yT_p[:])
                    y_p = psump.tile([P, C], f32, tag="y", bufs=1, name="y")
                    nc.tensor.transpose(y_p[:], yT[:], ident[:C, :C])
                    z = wp.tile([P, C], f32, tag="z", name="z")
                    nc.vector.tensor_tensor(z[:], y_p[:], b2_s[:],
                                            op=mybir.AluOpType.add)
                    negm = wp.tile([P, 1], f32, tag="negm", name="negm")
                    nc.vector.tensor_reduce(
                        negm[:], z[:], axis=mybir.AxisListType.X,
                        op=mybir.AluOpType.max, negate=True,
                    )
                    e = wp.tile([P, C], f32, tag="e", name="e")
                    sa = wp.tile([P, 1], f32, tag="sa", name="sa")
                    nc.scalar.activation(
                        e[:], z[:], mybir.ActivationFunctionType.Exp,
                        bias=negm[:], accum_out=sa[:],
                    )
                    lns = wp.tile([P, 1], f32, tag="lns", name="lns")
                    nc.scalar.activation(
                        lns[:], sa[:], mybir.ActivationFunctionType.Ln
                    )
                    o = wp.tile([P, C], f32, tag="o", name="o")
                    nc.vector.tensor_scalar(
                        out=o[:], in0=z[:], scalar1=negm[:], scalar2=lns[:],
                        op0=mybir.AluOpType.add, op1=mybir.AluOpType.subtract,
                    )
                    nc.sync.dma_start(outd[wi * P : (wi + 1) * P, :], o[:])

            _agg_pairs(nc, tc, mybir, table, idx_s, slot_s, iota_s, S, drain,
                       (gatp, selp, psump))
    nc.compile()
    return nc


def _run(nc, in_maps):
    if os.environ.get("BASS_GCN_SIM"):
        from concourse.bass_interp import MultiCoreSim

        sim = MultiCoreSim(nc, num_cores=NCORE, trace=False)
        for c in range(NCORE):
            for k, v in in_maps[c].items():
                sim.cores[c].tensor(k)[:] = v
        sim.simulate()
        outs = []
        for c in range(NCORE):
            names = [
                a.memorylocations[0].name
                for a in nc.m.functions[0].allocations
                if getattr(a, "kind", None) == "ExternalOutput"
            ]
            outs.append({n: np.array(sim.cores[c].tensor(n)) for n in names})
        return outs

    from concourse.bass_utils import run_bass_kernel_spmd

    trace = TRACE and _install_ntff_shim()
    res = run_bass_kernel_spmd(nc, in_maps, core_ids=list(range(NCORE)),
                               trace=trace)
    if res.exec_time_ns:
        LAST_EXEC_NS.append(res.exec_time_ns)
    return res.results


# ---------------------------------------------------------------- kernel
def kernel(x, edge_index, W1, b1, W2, b2):
    ncn, padn, nwin, npair, nchunk = _derived()
    LAST_EXEC_NS.clear()

    x = np.asarray(x, np.float32)
    edge_index = np.asarray(edge_index)
    W1 = np.asarray(W1, np.float32)
    b1 = np.asarray(b1, np.float32)
    W2 = np.asarray(W2, np.float32)
    b2 = np.asarray(b2, np.float32)

    plan = _build_plan(edge_index)
    S = plan["S"]
    idx_cols = plan["idxw"][0].shape[1]
    slot_cols = plan["slotcols"][0].shape[1]

    iota2 = np.tile(np.arange(2 * P, dtype=np.float32)[None, :], (P, 1))
    b1rep = np.tile(b1[None, :], (P, 1)).astype(np.float32)
    b2rep = np.tile(b2[None, :], (P, 1)).astype(np.float32)

    # ---- K1
    nc1 = _build_k1()
    in1 = []
    for c in range(NCORE):
        xc = np.zeros((padn, F_IN), np.float32)
        xc[:ncn] = x[c * ncn : (c + 1) * ncn]
        in1.append({
            "xT": np.ascontiguousarray(xc.T),
            "w1": W1,
            "dinvw": plan["dinv_w"][c],
        })
    r1 = _run(nc1, in1)
    utable = np.concatenate([r1[c]["ut"][:ncn] for c in range(NCORE)], axis=0)
    utable = np.ascontiguousarray(utable)

    # ---- K2
    nc2 = _build_k2(S, idx_cols, slot_cols)
    in2 = [{
        "table": utable,
        "idx": plan["idxw"][c],
        "slot": plan["slotcols"][c],
        "iota": iota2,
        "dinvw": plan["dinv_w"][c],
        "b1rep": b1rep,
    } for c in range(NCORE)]
    r2 = _run(nc2, in2)
    htable = np.concatenate([r2[c]["ht"][:ncn] for c in range(NCORE)], axis=0)
    htable = np.ascontiguousarray(htable)

    # ---- K3
    nc3 = _build_k3(S, idx_cols, slot_cols)
    in3 = [{
        "table": htable,
        "idx": plan["idxw"][c],
        "slot": plan["slotcols"][c],
        "iota": iota2,
        "dinvw": plan["dinv_w"][c],
        "w2": W2,
        "b2rep": b2rep,
    } for c in range(NCORE)]
    r3 = _run(nc3, in3)
    out = np.concatenate([r3[c]["out"][:ncn] for c in range(NCORE)], axis=0)
    return np.ascontiguousarray(out.astype(np.float32))

